# revision 4
# baseline (speedup 1.0000x reference)
"""CrossNet kernel for Trainium2 (8 NeuronCores, pure data parallel).

Math: reference computes, for l = 0..2:
    s_l = x_l . w_l   (per-row scalar)
    x_{l+1} = x0 * s_l + x_l + b_l

Unrolled (all dots are against x0):
    a_i   = x0 . w_i                     (per-row, i = 0..2)
    beta1 = b0 . w1,  beta2 = (b0+b1) . w2   (scalars)
    T3    = ((1+a0)(1+a1) + beta1)(1+a2) + beta2
    out   = x0 * T3 + (b0+b1+b2)

Per core (2048 rows), per 128-row tile:
  - DMA x tile [128, 1024] to SBUF
  - 8x PE transpose (128x128 chunks) -> PSUM, one ScalarE copy PSUM->SBUF
  - 8x PE fp32 matmul (lhsT = xT chunk, rhs = W^T chunk [128,3]) accumulating
    A = x @ W^T in PSUM [128, 3]
  - DVE: P = A + 1; T3 = P0*P1*P2 (plus beta terms when bias != 0)
  - ScalarE: out = x * T3 (per-partition scale)
  - DMA out
"""

import numpy as np

import concourse.bacc as bacc
import concourse.bass as bass
import concourse.mybir as mybir
import concourse.tile as tile
from concourse import masks
from concourse.bass_utils import run_bass_kernel_spmd

BATCH, DIM, LAYERS = 16384, 1024, 3
NCORES = 8
ROWS = BATCH // NCORES  # rows per core
P = 128                 # SBUF partitions
NT = ROWS // P          # row tiles per core
NCH = DIM // P          # 128-wide chunks of the feature dim

F32 = mybir.dt.float32


def _build(beta1: float, beta2: float, with_bias: bool):
    nc = bacc.Bacc("TRN2", target_bir_lowering=False, debug=False)

    x_d = nc.dram_tensor("x", [ROWS, DIM], F32, kind="ExternalInput").ap()
    wt_d = nc.dram_tensor("wt", [P, NCH * LAYERS], F32, kind="ExternalInput").ap()
    if with_bias:
        b3_d = nc.dram_tensor("b3", [P, DIM], F32, kind="ExternalInput").ap()
    out_d = nc.dram_tensor("out", [ROWS, DIM], F32, kind="ExternalOutput").ap()

    with tile.TileContext(nc) as tc:
        with (
            tc.tile_pool(name="const", bufs=1) as cpool,
            tc.tile_pool(name="xin", bufs=3) as xpool,
            tc.tile_pool(name="xt", bufs=2) as xtpool,
            tc.tile_pool(name="outp", bufs=3) as opool,
            tc.tile_pool(name="small", bufs=3) as spool,
            tc.tile_pool(name="psumT", bufs=2, space=bass.MemorySpace.PSUM) as ptpool,
            tc.tile_pool(name="psumA", bufs=2, space=bass.MemorySpace.PSUM) as papool,
        ):
            ident = cpool.tile([P, P], F32)
            masks.make_identity(nc, ident[:])

            # W^T chunks: wt[p, c*3 + i] = W[i, c*128 + p]
            wt_t = cpool.tile([P, NCH, LAYERS], F32)
            nc.sync.dma_start(wt_t[:], wt_d.rearrange("p (c l) -> p c l", l=LAYERS))

            if with_bias:
                b3_t = cpool.tile([P, DIM], F32)
                nc.sync.dma_start(b3_t[:], b3_d[:])

            for t in range(NT):
                xin = xpool.tile([P, DIM], F32)
                nc.sync.dma_start(xin[:], x_d[t * P:(t + 1) * P, :])

                # Transpose 128x128 chunks into PSUM, then one copy to SBUF.
                pT = ptpool.tile([P, DIM], F32)
                for k in range(NCH):
                    nc.tensor.transpose(
                        pT[:, k * P:(k + 1) * P], xin[:, k * P:(k + 1) * P], ident[:]
                    )
                xT = xtpool.tile([P, DIM], F32)
                nc.scalar.copy(xT[:], pT[:])

                # A[b, i] = sum_d x[b, d] * W[i, d], accumulated over chunks.
                pA = papool.tile([P, LAYERS], F32)
                for k in range(NCH):
                    nc.tensor.matmul(
                        pA[:],
                        xT[:, k * P:(k + 1) * P],
                        wt_t[:, k, :],
                        start=(k == 0),
                        stop=(k == NCH - 1),
                    )

                # T3 = ((1+a0)(1+a1)+beta1)(1+a2)+beta2
                pP = spool.tile([P, LAYERS], F32, tag="pP")
                nc.vector.tensor_scalar_add(pP[:], pA[:], 1.0)
                t2 = spool.tile([P, 1], F32, tag="t2")
                nc.vector.tensor_mul(t2[:], pP[:, 0:1], pP[:, 1:2])
                if beta1 != 0.0:
                    nc.vector.tensor_scalar_add(t2[:], t2[:], beta1)
                t3 = spool.tile([P, 1], F32, tag="t3")
                nc.vector.tensor_mul(t3[:], t2[:], pP[:, 2:3])
                if beta2 != 0.0:
                    nc.vector.tensor_scalar_add(t3[:], t3[:], beta2)

                xo = opool.tile([P, DIM], F32)
                if with_bias:
                    # out = x * T3 + B3 (one DVE op)
                    nc.vector.scalar_tensor_tensor(
                        xo[:], xin[:], t3[:], b3_t[:],
                        op0=mybir.AluOpType.mult, op1=mybir.AluOpType.add,
                    )
                else:
                    # out = x * T3 (ScalarE per-partition scale)
                    nc.scalar.mul(xo[:], xin[:], t3[:])

                nc.sync.dma_start(out_d[t * P:(t + 1) * P, :], xo[:])

    nc.compile()
    return nc


def kernel(x: np.ndarray, kernels: np.ndarray, bias: np.ndarray) -> np.ndarray:
    x = np.ascontiguousarray(x, dtype=np.float32)
    kernels = np.asarray(kernels, dtype=np.float32)
    bias = np.asarray(bias, dtype=np.float32)

    # Host-side tiny prep (O(LAYERS * DIM)): beta scalars and W^T layout.
    beta1 = float(bias[0] @ kernels[1])
    beta2 = float((bias[0] + bias[1]) @ kernels[2])
    b3 = bias.sum(axis=0)
    with_bias = bool(np.any(b3 != 0.0))

    # wt[p, c*3 + i] = W[i, c*128 + p] ; contiguous [128, 24] per-partition load
    wt = np.ascontiguousarray(
        kernels.T.reshape(NCH, P, LAYERS).transpose(1, 0, 2).reshape(P, NCH * LAYERS)
    )

    nc = _build(beta1, beta2, with_bias)

    in_maps = []
    for c in range(NCORES):
        m = {"x": x[c * ROWS:(c + 1) * ROWS], "wt": wt}
        if with_bias:
            m["b3"] = np.ascontiguousarray(np.broadcast_to(b3, (P, DIM)))
        in_maps.append(m)

    res = run_bass_kernel_spmd(nc, in_maps, list(range(NCORES)))
    return np.concatenate([r["out"] for r in res.results], axis=0)
